# revision 4
# baseline (speedup 1.0000x reference)
"""Enframe kernel for Trainium2 (Bass/Tile), SPMD over 8 NeuronCores.

Problem: x (16, 4, 160000) f32 -> out (16, 8192, 309) f32 where
  out[b, c*2048 + k, f] = x[b, c, 512*f + k]   (FRAME=2048, HOP=512, 309 frames)

Pure data movement. Per (b, c) slab view the signal as X2[j, r] = x[b, c, 512*j + r]
(j in [0,312), r in [0,512)). Then out[b, c*2048 + 512*q + r, f] = X2[f + q, r].
So the output is 4 shifted copies (q = 0..3) of the transpose of X2.

On-chip layout trick: the TensorE transpose input uses a stride-4 free-dim AP so
that SBUF partition p of the transposed tile holds output rows r = 4p..4p+3:
  T2[p, i, j] = X2[j, 4p + i]     (tile shape [128, 4, 312])
With that, the store for window-shift q is a single DMA whose HBM side is one
fully contiguous 632KB range (per-partition 4 adjacent 1236B rows), iterated
(p, i, f) with source T2[:, :, q:q+309].

Sharding: data-parallel over batch, 2 batches per core.
"""

import numpy as np

import concourse.bacc as bacc
import concourse.bass as bass
import concourse.mybir as mybir
import concourse.tile as tile
from concourse import masks
from concourse.bass_utils import run_bass_kernel_spmd

B, C, S = 16, 4, 160000
FRAME, HOP = 2048, 512
NF = (S - FRAME) // HOP + 1          # 309 frames
NBLK = NF + FRAME // HOP - 1         # 312 blocks of 512 samples actually used
N_CORES = 8
B_PER = B // N_CORES                 # 2 batches per core
F32 = mybir.dt.float32


def build_bass():
    nc = bacc.Bacc(None, target_bir_lowering=False)
    x = nc.dram_tensor("x", [B_PER, C, S], F32, kind="ExternalInput")
    out = nc.dram_tensor("out", [B_PER, C * FRAME, NF], F32, kind="ExternalOutput")

    with tile.TileContext(nc) as tc:
        with (
            tc.tile_pool(name="singles", bufs=1) as singles,
            tc.tile_pool(name="a", bufs=6) as a_pool,
            tc.tile_pool(name="t2", bufs=3) as t2_pool,
            tc.tile_pool(name="ps", bufs=8, space=bass.MemorySpace.PSUM) as ps_pool,
        ):
            ident = singles.tile([128, 128], F32)
            masks.make_identity(nc, ident[:])

            for b in range(B_PER):
                for c in range(C):
                    slab_off = (b * C + c) * S
                    # T2[p, i, j] = X2[j, 4p + i]
                    t2 = t2_pool.tile([128, 4, NBLK], F32)
                    for jt in range(3):
                        pj = 128 if jt < 2 else NBLK - 256  # 128, 128, 56
                        # A3[pp, f, i] = X2[jt*128 + pp, 4f + i]; load is one
                        # contiguous 2KB-per-partition DMA.
                        a3 = a_pool.tile([128, 128, 4], F32)
                        src = bass.AP(x, slab_off + jt * 128 * HOP,
                                      [[HOP, pj], [4, 128], [1, 4]])
                        nc.sync.dma_start(out=a3[:pj], in_=src)
                        for i in range(4):
                            pst = ps_pool.tile([128, 128], F32)
                            nc.tensor.transpose(
                                pst[:, :pj], a3[:pj, :, i], ident[:pj, :pj]
                            )
                            nc.any.tensor_copy(
                                out=t2[:, i, jt * 128 : jt * 128 + pj],
                                in_=pst[:, :pj],
                            )
                    for q in range(4):
                        # out rows c*2048 + 512q + 4p + i, all 309 frames:
                        # one contiguous 632,832B HBM range.
                        dst = bass.AP(
                            out,
                            (b * C * FRAME + c * FRAME + q * HOP) * NF,
                            [[4 * NF, 128], [NF, 4], [1, NF]],
                        )
                        nc.sync.dma_start(out=dst, in_=t2[:, :, q : q + NF])
    nc.finalize()
    return nc


_NC_CACHE = None


def kernel(x: np.ndarray) -> np.ndarray:
    global _NC_CACHE
    if _NC_CACHE is None:
        _NC_CACHE = build_bass()
    nc = _NC_CACHE
    in_maps = [
        {"x": np.ascontiguousarray(x[i * B_PER : (i + 1) * B_PER])}
        for i in range(N_CORES)
    ]
    res = run_bass_kernel_spmd(nc, in_maps, list(range(N_CORES)))
    return np.concatenate([r["out"] for r in res.results], axis=0)


# revision 6
# speedup vs baseline: 1.3878x; 1.3878x over previous
"""Enframe kernel for Trainium2 (Bass/Tile), SPMD over 8 NeuronCores.

Problem: x (16, 4, 160000) f32 -> out (16, 8192, 309) f32 where
  out[b, c*2048 + k, f] = x[b, c, 512*f + k]   (FRAME=2048, HOP=512, 309 frames)

Pure data movement. Per (b, c) slab view the signal as X2[j, r] = x[b, c, 512*j + r]
(j in [0,312), r in [0,512)). Then out[b, c*2048 + 512*q + r, f] = X2[f + q, r].
So the output is 4 shifted copies (q = 0..3) of the transpose of X2.

On-chip layout trick: the TensorE transpose input uses a stride-4 free-dim AP so
that SBUF partition p of the transposed tile holds output rows r = 4p..4p+3:
  T2[p, i, j] = X2[j, 4p + i]     (tile shape [128, 4, 312])
With that, the store for window-shift q is a single DMA whose HBM side is one
fully contiguous 632KB range (per-partition 4 adjacent 1236B rows), iterated
(p, i, f) with source T2[:, :, q:q+309].

Sharding: data-parallel over batch, 2 batches per core.
"""

import numpy as np

import concourse.bacc as bacc
import concourse.bass as bass
import concourse.mybir as mybir
import concourse.tile as tile
from concourse import masks
from concourse.bass_utils import run_bass_kernel_spmd

B, C, S = 16, 4, 160000
FRAME, HOP = 2048, 512
NF = (S - FRAME) // HOP + 1          # 309 frames
NBLK = NF + FRAME // HOP - 1         # 312 blocks of 512 samples actually used
N_CORES = 8
B_PER = B // N_CORES                 # 2 batches per core
F32 = mybir.dt.float32


def build_bass():
    nc = bacc.Bacc(None, target_bir_lowering=False)
    x = nc.dram_tensor("x", [B_PER, C, S], F32, kind="ExternalInput")
    out = nc.dram_tensor("out", [B_PER, C * FRAME, NF], F32, kind="ExternalOutput")

    with tile.TileContext(nc) as tc:
        with (
            tc.tile_pool(name="singles", bufs=1) as singles,
            tc.tile_pool(name="a", bufs=6) as a_pool,
            tc.tile_pool(name="t2", bufs=3) as t2_pool,
            tc.tile_pool(name="oq", bufs=8) as oq_pool,
            tc.tile_pool(name="ps", bufs=8, space=bass.MemorySpace.PSUM) as ps_pool,
        ):
            ident = singles.tile([128, 128], F32)
            masks.make_identity(nc, ident[:])

            for b in range(B_PER):
                for c in range(C):
                    slab_off = (b * C + c) * S
                    # T2[p, i, j] = X2[j, 4p + i]
                    t2 = t2_pool.tile([128, 4, NBLK], F32)
                    for jt in range(3):
                        pj = 128 if jt < 2 else NBLK - 256  # 128, 128, 56
                        # A3[pp, f, i] = X2[jt*128 + pp, 4f + i]; load is one
                        # contiguous 2KB-per-partition DMA (on the ACT HWDGE
                        # queue so load dispatch doesn't serialize stores).
                        a3 = a_pool.tile([128, 128, 4], F32)
                        src = bass.AP(x, slab_off + jt * 128 * HOP,
                                      [[HOP, pj], [4, 128], [1, 4]])
                        nc.scalar.dma_start(out=a3[:pj], in_=src)
                        for i in range(4):
                            pst = ps_pool.tile([128, 128], F32)
                            nc.tensor.transpose(
                                pst[:, :pj], a3[:pj, :, i], ident[:pj, :pj]
                            )
                            nc.scalar.copy(
                                out=t2[:, i, jt * 128 : jt * 128 + pj],
                                in_=pst[:, :pj],
                            )
                    for q in range(4):
                        # Materialize the q-shifted window in exact output
                        # layout so the store's SBUF-side runs are 4944B
                        # contiguous per partition (big DMA descriptors).
                        oq = oq_pool.tile([128, 4, NF], F32)
                        nc.vector.tensor_copy(out=oq[:], in_=t2[:, :, q : q + NF])
                        # out rows c*2048 + 512q + 4p + i, all 309 frames:
                        # one contiguous 632,832B HBM range.
                        dst = bass.AP(
                            out,
                            (b * C * FRAME + c * FRAME + q * HOP) * NF,
                            [[4 * NF, 128], [NF, 4], [1, NF]],
                        )
                        nc.sync.dma_start(out=dst, in_=oq[:])
    nc.finalize()
    return nc


_NC_CACHE = None


def kernel(x: np.ndarray) -> np.ndarray:
    global _NC_CACHE
    if _NC_CACHE is None:
        _NC_CACHE = build_bass()
    nc = _NC_CACHE
    in_maps = [
        {"x": np.ascontiguousarray(x[i * B_PER : (i + 1) * B_PER])}
        for i in range(N_CORES)
    ]
    res = run_bass_kernel_spmd(nc, in_maps, list(range(N_CORES)))
    return np.concatenate([r["out"] for r in res.results], axis=0)


# revision 9
# speedup vs baseline: 1.4282x; 1.0291x over previous
"""Enframe kernel for Trainium2 (Bass/Tile), SPMD over 8 NeuronCores.

Problem: x (16, 4, 160000) f32 -> out (16, 8192, 309) f32 where
  out[b, c*2048 + k, f] = x[b, c, 512*f + k]   (FRAME=2048, HOP=512, 309 frames)

Pure data movement. Per (b, c) slab view the signal as X2[j, r] = x[b, c, 512*j + r]
(j in [0,312), r in [0,512)). Then out[b, c*2048 + 512*q + r, f] = X2[f + q, r].
So the output is 4 shifted copies (q = 0..3) of the transpose of X2.

On-chip layout trick: the TensorE transpose input uses a stride-4 free-dim AP so
that SBUF partition p of the transposed tile holds output rows r = 4p..4p+3:
  T2[p, i, j] = X2[j, 4p + i]     (tile shape [128, 4, 312])
With that, the store for window-shift q is a single DMA whose HBM side is one
fully contiguous 632KB range (per-partition 4 adjacent 1236B rows), iterated
(p, i, f) with source T2[:, :, q:q+309].

Sharding: data-parallel over batch, 2 batches per core.
"""

import numpy as np

import concourse.bacc as bacc
import concourse.bass as bass
import concourse.mybir as mybir
import concourse.tile as tile
from concourse import masks
from concourse.bass_utils import run_bass_kernel_spmd

B, C, S = 16, 4, 160000
FRAME, HOP = 2048, 512
NF = (S - FRAME) // HOP + 1          # 309 frames
NBLK = NF + FRAME // HOP - 1         # 312 blocks of 512 samples actually used
N_CORES = 8
B_PER = B // N_CORES                 # 2 batches per core
F32 = mybir.dt.float32


def build_bass():
    nc = bacc.Bacc(None, target_bir_lowering=False)
    x = nc.dram_tensor("x", [B_PER, C, S], F32, kind="ExternalInput")
    out = nc.dram_tensor("out", [B_PER, C * FRAME, NF], F32, kind="ExternalOutput")

    with tile.TileContext(nc) as tc:
        with (
            tc.tile_pool(name="singles", bufs=1) as singles,
            tc.tile_pool(name="a", bufs=8) as a_pool,
            tc.tile_pool(name="t2", bufs=4) as t2_pool,
            tc.tile_pool(name="oq", bufs=12) as oq_pool,
            tc.tile_pool(name="ps", bufs=8, space=bass.MemorySpace.PSUM) as ps_pool,
        ):
            ident = singles.tile([128, 128], F32)
            masks.make_identity(nc, ident[:])

            for b in range(B_PER):
                for c in range(C):
                    slab_off = (b * C + c) * S
                    # T2[p, i, j] = X2[j, 4p + i]
                    t2 = t2_pool.tile([128, 4, NBLK], F32)
                    for jt in range(3):
                        pj = 128 if jt < 2 else NBLK - 256  # 128, 128, 56
                        # A3[pp, f, i] = X2[jt*128 + pp, 4f + i]; load is one
                        # contiguous 2KB-per-partition DMA (on the idle SWDGE
                        # queue so load dispatch doesn't serialize copies).
                        a3 = a_pool.tile([128, 128, 4], F32)
                        src = bass.AP(x, slab_off + jt * 128 * HOP,
                                      [[HOP, pj], [4, 128], [1, 4]])
                        nc.gpsimd.dma_start(out=a3[:pj], in_=src)
                        for i in range(4):
                            pst = ps_pool.tile([128, 128], F32)
                            nc.tensor.transpose(
                                pst[:, :pj], a3[:pj, :, i], ident[:pj, :pj]
                            )
                            if i < 2:
                                nc.vector.tensor_copy(
                                    out=t2[:, i, jt * 128 : jt * 128 + pj],
                                    in_=pst[:, :pj],
                                )
                            else:
                                nc.scalar.copy(
                                    out=t2[:, i, jt * 128 : jt * 128 + pj],
                                    in_=pst[:, :pj],
                                )
                    for q in range(4):
                        # Materialize the q-shifted window in exact output
                        # layout so the store's SBUF-side runs are 4944B
                        # contiguous per partition (big DMA descriptors).
                        oq = oq_pool.tile([128, 4, NF], F32)
                        if q < 2:
                            nc.vector.tensor_copy(out=oq[:], in_=t2[:, :, q : q + NF])
                        else:
                            nc.scalar.copy(out=oq[:], in_=t2[:, :, q : q + NF])
                        # out rows c*2048 + 512q + 4p + i, all 309 frames:
                        # one contiguous 632,832B HBM range.
                        dst = bass.AP(
                            out,
                            (b * C * FRAME + c * FRAME + q * HOP) * NF,
                            [[4 * NF, 128], [NF, 4], [1, NF]],
                        )
                        nc.sync.dma_start(out=dst, in_=oq[:])
    nc.finalize()
    return nc


_NC_CACHE = None


def kernel(x: np.ndarray) -> np.ndarray:
    global _NC_CACHE
    if _NC_CACHE is None:
        _NC_CACHE = build_bass()
    nc = _NC_CACHE
    in_maps = [
        {"x": np.ascontiguousarray(x[i * B_PER : (i + 1) * B_PER])}
        for i in range(N_CORES)
    ]
    res = run_bass_kernel_spmd(nc, in_maps, list(range(N_CORES)))
    return np.concatenate([r["out"] for r in res.results], axis=0)
